# revision 4
# baseline (speedup 1.0000x reference)
"""Trainium2 Bass kernel for nn_MultiHeadAttention_79130477461654.

The reference einsum "nhqk,nhvd->nhqd" contracts k and v independently, so
out = (sum_k softmax(energy))*(sum_s v) = broadcast(sum_s v) since softmax
rows sum to 1.  With v = split_heads(x @ Wv) and the reference's direct
(n,h,q,d)->(n,s,e) reshape, the full output reduces to

    xs[n]    = sum_s x[n,s,:]                       (1024,)
    Z[n]     = xs[n] @ Wv                           (1024,)
    WoSum    = sum_m Wo[64m+d, :]  (d=0..63)        (64, 1024)
    T[n,h,:] = Z[n][64h:64h+64] @ WoSum + bo        (16, 1024)
    out[n, 64h+r, :] = T[n,h,:]   for r in 0..63

Sharding: data parallel over batch N=8, one batch per core; Wv/Wo
replicated.  All arithmetic on-device.

This version vs the previous one:
  - x is passed pre-transposed (e, s) so the seq-sum is a DVE free-dim
    tensor_reduce (no PE partition reduction, no pairwise adds).
  - Wv/Wo DRAM params are declared float32r (same bits as f32), killing
    all of the DVE cast traffic.
  - WoSum is kept 128-partition-folded (w128) and the T matmul uses a
    duplicated lhsT (YTx = [sft; sft]) so no cross-partition fold needed.
  - Output is written as bf16 via one broadcast-DMA (stride-0 repeat dim),
    halving the output write phase.
"""

import numpy as np

N, S, E, H, D = 8, 1024, 1024, 16, 64
NCORES = 8
P = 128  # partitions

OUT_BF16 = True


def build_nc():
    import concourse.bacc as bacc
    import concourse.mybir as mybir
    from concourse.tile import TileContext

    F32 = mybir.dt.float32
    F32R = mybir.dt.float32r
    BF16 = mybir.dt.bfloat16
    ODT = BF16 if OUT_BF16 else F32
    nc = bacc.Bacc("TRN2", target_bir_lowering=False, debug=False)

    xtd = nc.declare_dram_parameter("xT", [E, S], F32, isOutput=False)
    wvd = nc.declare_dram_parameter("Wv", [E, E], F32R, isOutput=False)
    wod = nc.declare_dram_parameter("Wo", [E, E], F32R, isOutput=False)
    bod = nc.declare_dram_parameter("bo128", [P, E], F32, isOutput=False)
    i2d = nc.declare_dram_parameter("I2", [D, P], F32, isOutput=False)
    oned = nc.declare_dram_parameter("one1", [1, 1], F32, isOutput=False)
    outd = nc.declare_dram_parameter("out", [S, E], ODT, isOutput=True)

    # two HWDGE queues: SP (sync) and ACT (scalar)
    dmae = [nc.sync, nc.scalar]

    # DRAM-side views pairing two 128-row chunks per 1 MB transfer:
    # paired(src, i)[p, c, :] = src[(2i + c)*128 + p, :]
    def paired(dram, i):
        return dram.rearrange("(i c p) e -> i p c e", p=P, c=2)[i]

    with TileContext(nc) as tc:
        with (
            tc.tile_pool(name="xin", bufs=4) as xp,
            tc.tile_pool(name="wv", bufs=4) as wvp,
            tc.tile_pool(name="wo", bufs=4) as wop,
            tc.tile_pool(name="small", bufs=1) as sp,
            tc.tile_pool(name="psZ", bufs=1, space="PSUM") as psZ,
            tc.tile_pool(name="psS", bufs=1, space="PSUM") as psS,
            tc.tile_pool(name="psY", bufs=1, space="PSUM") as psY,
            tc.tile_pool(name="psT", bufs=1, space="PSUM") as psT,
        ):
            # tiny consts on the SWDGE queue so the HWDGE queues stream x at once
            one_sb = sp.tile([1, 1], F32)
            nc.gpsimd.dma_start(out=one_sb[:], in_=oned[:])
            i2_sb = sp.tile([D, P], F32)
            nc.gpsimd.dma_start(out=i2_sb[:], in_=i2d[:])
            bo_sb = sp.tile([P, E], F32)
            nc.gpsimd.dma_start(out=bo_sb[:], in_=bod[:])

            # ---- input DMAs: xT, Wv, Wo as 1 MB paired transfers
            xpT = sp.tile([P, 8], F32R)  # xpT[p, k] = sum_s x[128k+p, s]
            xts, wvt, wot = [], [], []
            for i in range(4):
                t = xp.tile([P, 2 * S], F32, tag="xt")
                dmae[i % 2].dma_start(
                    out=t[:].rearrange("p (c s) -> p c s", c=2), in_=paired(xtd, i)
                )
                xts.append(t)
            for i in range(4):
                t = wvp.tile([P, 2 * E], F32R, tag="wv")
                dmae[i % 2].dma_start(
                    out=t[:].rearrange("p (c e) -> p c e", c=2), in_=paired(wvd, i)
                )
                wvt.append(t)
            for i in range(4):
                t = wop.tile([P, 2 * E], F32R, tag="wo")
                dmae[i % 2].dma_start(
                    out=t[:].rearrange("p (c e) -> p c e", c=2), in_=paired(wod, i)
                )
                wot.append(t)

            # ---- DVE work in DMA arrival order (DVE is FIFO): x reduces,
            #      then Wo folds, then the z-dependent tail.
            with nc.allow_low_precision("f32r accumulate is full fp32 on DVE"):
                for i in range(4):
                    nc.vector.tensor_reduce(
                        xpT[:, 2 * i : 2 * i + 2],
                        xts[i][:].rearrange("p (c s) -> p c s", c=2),
                        axis=mybir.AxisListType.X,
                        op=mybir.AluOpType.add,
                    )

            # w128[p, :] = sum_rb Wo[128 rb + p, :]: DVE folds chasing Wo
            w128 = sp.tile([P, E], F32R)
            nc.vector.tensor_add(w128[:], wot[0][:, 0:E], wot[0][:, E : 2 * E])
            for i in range(1, 4):
                tmp = wop.tile([P, E], F32R, tag="wps")
                nc.vector.tensor_add(tmp[:], wot[i][:, 0:E], wot[i][:, E : 2 * E])
                nc.vector.tensor_add(w128[:], w128[:], tmp[:])

            # ---- Z row (1, 1024) = xs @ Wv  (wide fp32r, chases Wv DMA)
            ps_z = psZ.tile([1, E], F32, tag="psz")
            for k in range(8):
                base = (k % 2) * E
                for half in range(2):
                    sl = slice(half * 512, half * 512 + 512)
                    nc.tensor.matmul(
                        ps_z[0:1, sl],
                        xpT[:, k : k + 1],
                        wvt[k // 2][:, base + half * 512 : base + half * 512 + 512],
                        start=(k == 0),
                        stop=(k == 7),
                        skip_group_check=True,
                    )
            srow = sp.tile([1, E], F32)
            nc.vector.tensor_copy(srow[:], ps_z[:])

            # ---- sft[d, h] = Z[64h + d]  (rank-1 matmuls, K=1)
            ps_sft = psS.tile([D, H], F32, tag="pss")
            for h in range(H):
                nc.tensor.matmul(
                    ps_sft[:, h : h + 1],
                    srow[0:1, h * D : (h + 1) * D],
                    one_sb[0:1, 0:1],
                    start=True,
                    stop=True,
                )
            # sft8[d, 8h + rr] = sft[d, h]  (free-dim broadcast to 128 cols)
            sft8 = sp.tile([D, P], F32)
            nc.vector.tensor_copy(
                sft8[:].rearrange("d (h rr) -> d h rr", rr=8),
                ps_sft[:, :, None].to_broadcast((D, H, 8)),
            )

            # ---- YTx8 (128, 128) = [sft8; sft8] via dup matmul
            #      (I2[k,m]=1 iff m%64==k), rows m = 8h + rr
            ps_ytx = psY.tile([P, P], F32, tag="psy")
            nc.tensor.matmul(ps_ytx[:], i2_sb[:], sft8[:], start=True, stop=True)
            ytx8 = sp.tile([P, P], F32R)
            nc.vector.tensor_copy(ytx8[:], ps_ytx[:])

            # ---- T8 (128, 1024) = YTx8.T @ w128  (fp32r full rate at N=512)
            ps_t = psT.tile([P, E], F32, tag="pst")
            for half in range(2):
                sl = slice(half * 512, half * 512 + 512)
                nc.tensor.matmul(
                    ps_t[:, sl], ytx8[:], w128[:, sl], start=True, stop=True,
                    skip_group_check=True,
                )
            tb8 = sp.tile([P, E], mybir.dt.bfloat16 if OUT_BF16 else F32)
            nc.vector.tensor_add(tb8[:], ps_t[:], bo_sb[:])

            # ---- broadcast store: out[8m + r8, :] = T8[m, :] = T[m//8, :],
            #      one DMA from all 128 partitions
            dmae[0].dma_start(
                out=outd.rearrange("(m r8) j -> m r8 j", r8=8),
                in_=tb8[:, None, :].to_broadcast((P, 8, E)),
            )

    nc.compile()
    return nc


_NC_CACHE = None


def make_in_maps(x, Wv, Wo, bo):
    x = np.ascontiguousarray(np.asarray(x, dtype=np.float32))
    Wv = np.ascontiguousarray(np.asarray(Wv, dtype=np.float32))
    Wo = np.ascontiguousarray(np.asarray(Wo, dtype=np.float32))
    bo = np.ascontiguousarray(np.asarray(bo, dtype=np.float32))
    bo16 = np.tile(bo.reshape(1, E), (H, 1))
    I2 = np.zeros((D, P), dtype=np.float32)
    I2[np.arange(P) % D, np.arange(P)] = 1.0
    one1 = np.ones((1, 1), dtype=np.float32)
    return [
        {
            "xT": np.ascontiguousarray(x[j].T),
            "Wv": Wv,
            "Wo": Wo,
            "bo16": bo16,
            "I2": I2,
            "one1": one1,
        }
        for j in range(NCORES)
    ]


def kernel(x, Wq=None, Wk=None, Wv=None, Wo=None, bo=None, **_unused):
    from concourse.bass_utils import run_bass_kernel_spmd

    global _NC_CACHE
    if _NC_CACHE is None:
        _NC_CACHE = build_nc()
    nc = _NC_CACHE

    in_maps = make_in_maps(x, Wv, Wo, bo)
    res = run_bass_kernel_spmd(nc, in_maps, core_ids=list(range(NCORES))).results
    return np.stack(
        [res[j]["out"].astype(np.float32) for j in range(NCORES)], axis=0
    )


# revision 6
# speedup vs baseline: 1.7628x; 1.7628x over previous
"""Trainium2 Bass kernel for nn_MultiHeadAttention_79130477461654.

The reference einsum "nhqk,nhvd->nhqd" contracts k and v independently, so
out = (sum_k softmax(energy))*(sum_s v) = broadcast(sum_s v) since softmax
rows sum to 1.  With v = split_heads(x @ Wv) and the reference's direct
(n,h,q,d)->(n,s,e) reshape, the full output reduces to

    xs[n]    = sum_s x[n,s,:]                       (1024,)
    Z[n]     = xs[n] @ Wv                           (1024,)
    WoSum    = sum_m Wo[64m+d, :]  (d=0..63)        (64, 1024)
    T[n,h,:] = Z[n][64h:64h+64] @ WoSum + bo        (16, 1024)
    out[n, 64h+r, :] = T[n,h,:]   for r in 0..63

Sharding: data parallel over batch N=8, one batch per core; Wv/Wo
replicated.  All arithmetic on-device.

This version vs the previous one:
  - x is passed pre-transposed (e, s) so the seq-sum is a DVE free-dim
    tensor_reduce (no PE partition reduction, no pairwise adds).
  - Wv/Wo DRAM params are declared float32r (same bits as f32), killing
    all of the DVE cast traffic.
  - WoSum is kept 128-partition-folded (w128) and the T matmul uses a
    duplicated lhsT (YTx = [sft; sft]) so no cross-partition fold needed.
  - Output is written as bf16 via one broadcast-DMA (stride-0 repeat dim),
    halving the output write phase.
"""

import numpy as np

N, S, E, H, D = 8, 1024, 1024, 16, 64
NCORES = 8
P = 128  # partitions

OUT_BF16 = True


def build_nc():
    import concourse.bacc as bacc
    import concourse.mybir as mybir
    from concourse.tile import TileContext

    F32 = mybir.dt.float32
    F32R = mybir.dt.float32r
    BF16 = mybir.dt.bfloat16
    ODT = BF16 if OUT_BF16 else F32
    nc = bacc.Bacc("TRN2", target_bir_lowering=False, debug=False)

    xtd = nc.declare_dram_parameter("xT", [E, S], F32, isOutput=False)
    wvd = nc.declare_dram_parameter("Wv", [E, E], F32R, isOutput=False)
    wod = nc.declare_dram_parameter("Wo", [E, E], F32R, isOutput=False)
    bod = nc.declare_dram_parameter("bo128", [P, E], F32, isOutput=False)
    i2d = nc.declare_dram_parameter("I2", [D, P], F32, isOutput=False)
    oned = nc.declare_dram_parameter("one1", [1, 1], F32, isOutput=False)
    outd = nc.declare_dram_parameter("out", [S, E], ODT, isOutput=True)

    # two HWDGE queues: SP (sync) and ACT (scalar)
    dmae = [nc.sync, nc.scalar]

    # DRAM-side views pairing two 128-row chunks per 1 MB transfer:
    # paired(src, i)[p, c, :] = src[(2i + c)*128 + p, :]
    def paired(dram, i):
        return dram.rearrange("(i c p) e -> i p c e", p=P, c=2)[i]

    with TileContext(nc) as tc:
        with (
            tc.tile_pool(name="xin", bufs=4) as xp,
            tc.tile_pool(name="wv", bufs=4) as wvp,
            tc.tile_pool(name="wo", bufs=4) as wop,
            tc.tile_pool(name="small", bufs=1) as sp,
            tc.tile_pool(name="psZ", bufs=1, space="PSUM") as psZ,
            tc.tile_pool(name="psS", bufs=1, space="PSUM") as psS,
            tc.tile_pool(name="psY", bufs=1, space="PSUM") as psY,
            tc.tile_pool(name="psT", bufs=1, space="PSUM") as psT,
        ):
            # tiny consts on the SWDGE queue so the HWDGE queues stream x at once
            one_sb = sp.tile([1, 1], F32)
            nc.gpsimd.dma_start(out=one_sb[:], in_=oned[:])
            i2_sb = sp.tile([D, P], F32)
            nc.gpsimd.dma_start(out=i2_sb[:], in_=i2d[:])
            bo_sb = sp.tile([P, E], F32)
            nc.gpsimd.dma_start(out=bo_sb[:], in_=bod[:])

            # ---- input DMAs: xT, Wv, Wo as 1 MB paired transfers
            xpT = sp.tile([P, 8], F32R)  # xpT[p, k] = sum_s x[128k+p, s]
            xts, wvt, wot = [], [], []
            for i in range(4):
                t = xp.tile([P, 2 * S], F32, tag="xt")
                dmae[i % 2].dma_start(
                    out=t[:].rearrange("p (c s) -> p c s", c=2), in_=paired(xtd, i)
                )
                xts.append(t)
            for i in range(4):
                t = wvp.tile([P, 2 * E], F32R, tag="wv")
                dmae[i % 2].dma_start(
                    out=t[:].rearrange("p (c e) -> p c e", c=2), in_=paired(wvd, i)
                )
                wvt.append(t)
            for i in range(4):
                t = wop.tile([P, 2 * E], F32R, tag="wo")
                dmae[i % 2].dma_start(
                    out=t[:].rearrange("p (c e) -> p c e", c=2), in_=paired(wod, i)
                )
                wot.append(t)

            # ---- DVE work in DMA arrival order (DVE is FIFO): x reduces,
            #      then Wo folds, then the z-dependent tail.
            with nc.allow_low_precision("f32r accumulate is full fp32 on DVE"):
                for i in range(4):
                    nc.vector.tensor_reduce(
                        xpT[:, 2 * i : 2 * i + 2],
                        xts[i][:].rearrange("p (c s) -> p c s", c=2),
                        axis=mybir.AxisListType.X,
                        op=mybir.AluOpType.add,
                    )

            # w128[p, :] = sum_rb Wo[128 rb + p, :]: DVE folds chasing Wo
            w128 = sp.tile([P, E], F32R)
            nc.vector.tensor_add(w128[:], wot[0][:, 0:E], wot[0][:, E : 2 * E])
            for i in range(1, 4):
                tmp = wop.tile([P, E], F32R, tag="wps")
                nc.vector.tensor_add(tmp[:], wot[i][:, 0:E], wot[i][:, E : 2 * E])
                nc.vector.tensor_add(w128[:], w128[:], tmp[:])

            # ---- Z row (1, 1024) = xs @ Wv  (wide fp32r, chases Wv DMA)
            ps_z = psZ.tile([1, E], F32, tag="psz")
            for k in range(8):
                base = (k % 2) * E
                for half in range(2):
                    sl = slice(half * 512, half * 512 + 512)
                    nc.tensor.matmul(
                        ps_z[0:1, sl],
                        xpT[:, k : k + 1],
                        wvt[k // 2][:, base + half * 512 : base + half * 512 + 512],
                        start=(k == 0),
                        stop=(k == 7),
                        skip_group_check=True,
                    )
            srow = sp.tile([1, E], F32)
            nc.vector.tensor_copy(srow[:], ps_z[:])

            # ---- sft[d, h] = Z[64h + d]  (rank-1 matmuls, K=1)
            ps_sft = psS.tile([D, H], F32, tag="pss")
            for h in range(H):
                nc.tensor.matmul(
                    ps_sft[:, h : h + 1],
                    srow[0:1, h * D : (h + 1) * D],
                    one_sb[0:1, 0:1],
                    start=True,
                    stop=True,
                )
            # sft8[d, 8h + rr] = sft[d, h]  (free-dim broadcast to 128 cols)
            sft8 = sp.tile([D, P], F32)
            nc.vector.tensor_copy(
                sft8[:].rearrange("d (h rr) -> d h rr", rr=8),
                ps_sft[:, :, None].to_broadcast((D, H, 8)),
            )

            # ---- YTx8 (128, 128) = [sft8; sft8] via dup matmul
            #      (I2[k,m]=1 iff m%64==k), rows m = 8h + rr
            ps_ytx = psY.tile([P, P], F32, tag="psy")
            nc.tensor.matmul(ps_ytx[:], i2_sb[:], sft8[:], start=True, stop=True)
            ytx8 = sp.tile([P, P], F32R)
            nc.vector.tensor_copy(ytx8[:], ps_ytx[:])

            # ---- T8 (128, 1024) = YTx8.T @ w128  (fp32r full rate at N=512)
            ps_t = psT.tile([P, E], F32, tag="pst")
            for half in range(2):
                sl = slice(half * 512, half * 512 + 512)
                nc.tensor.matmul(
                    ps_t[:, sl], ytx8[:], w128[:, sl], start=True, stop=True,
                    skip_group_check=True,
                )
            tb8 = sp.tile([P, E], mybir.dt.bfloat16 if OUT_BF16 else F32)
            nc.vector.tensor_add(tb8[:], ps_t[:], bo_sb[:])

            # ---- broadcast store: out[8m + r8, :] = T8[m, :] = T[m//8, :],
            #      one DMA from all 128 partitions
            dmae[0].dma_start(
                out=outd.rearrange("(m r8) j -> m r8 j", r8=8),
                in_=tb8[:, None, :].to_broadcast((P, 8, E)),
            )

    nc.compile()
    return nc


_NC_CACHE = None


def make_in_maps(x, Wv, Wo, bo):
    x = np.ascontiguousarray(np.asarray(x, dtype=np.float32))
    Wv = np.ascontiguousarray(np.asarray(Wv, dtype=np.float32))
    Wo = np.ascontiguousarray(np.asarray(Wo, dtype=np.float32))
    bo = np.ascontiguousarray(np.asarray(bo, dtype=np.float32))
    bo128 = np.tile(bo.reshape(1, E), (P, 1))
    I2 = np.zeros((D, P), dtype=np.float32)
    I2[np.arange(P) % D, np.arange(P)] = 1.0
    one1 = np.ones((1, 1), dtype=np.float32)
    return [
        {
            "xT": np.ascontiguousarray(x[j].T),
            "Wv": Wv,
            "Wo": Wo,
            "bo128": bo128,
            "I2": I2,
            "one1": one1,
        }
        for j in range(NCORES)
    ]


def kernel(x, Wq=None, Wk=None, Wv=None, Wo=None, bo=None, **_unused):
    from concourse.bass_utils import run_bass_kernel_spmd

    global _NC_CACHE
    if _NC_CACHE is None:
        _NC_CACHE = build_nc()
    nc = _NC_CACHE

    in_maps = make_in_maps(x, Wv, Wo, bo)
    res = run_bass_kernel_spmd(nc, in_maps, core_ids=list(range(NCORES))).results
    return np.stack(
        [res[j]["out"].astype(np.float32) for j in range(NCORES)], axis=0
    )
